# revision 41
# baseline (speedup 1.0000x reference)
"""Trainium2 Bass kernel for the 3x3 local-attention module (sparse_attention).

Sharding: 8 cores = (batch b in 0..3) x (out-channel half in 0..1).
Each core computes one batch element and 64 of the 128 output channels
(= 256 of the 512 OG channels). No cross-core communication; GroupNorm
statistics are exact per (b, out) and live entirely on one core.

Math restructure (validated in numpy against the jax reference, ~4e-3 rel l2):
  ke  = relu(Wk' x + sh1)                  (BN folded into Wk)
  h_p = relu(W1a x + W1b shift_p(ke) + b1')   via PSUM accumulation
  a0  = W2 h                               (no bias; b2 handled analytically)
  stats: S1 = sum a0, S2 = sum a0^2 per og; group mean/var derived with b2.
  AV[out,hw] = sum_{gc,p} gn_g * a0 * shift_p(v)   (products on DVE/GPSIMD,
               gc-sum + gn_g weights + p-sum via one PE matmul per (chunk,p))
  const term = box3x3( sum_gc w2[og]*v[og] ),  w2 = gn_g*(b2-mu)*rstd + gn_b
  out = rstd * AV + const
"""

import numpy as np

B, C, H, W = 4, 128, 56, 56
HW = H * W
K2 = 9
MID = 32
OG = 512
COUT = 128
GC = 4
EPS = 1e-5
NT = 7                # N-tiles over HW
TW = HW // NT         # 448 = 8 rows of 56
TR = TW // W          # 8 rows per tile
PADW = W + 2          # 58
PADHW = PADW * PADW   # 3364
NSTAT = K2 * HW       # per-og element count for stats
DELTA = [(i - 1, j - 1) for i in range(3) for j in range(3)]
PGROUPS = [[0, 1, 2, 3], [4, 5, 6, 7], [8]]


def _f32(x):
    return np.ascontiguousarray(x, dtype=np.float32)


def _make_w1btz(W1b):
    z = np.zeros((128, 4, 128), np.float32)
    for j in range(4):
        z[:, j, j * 32:(j + 1) * 32] = W1b.T
    return z


def _prep_weights(inp):
    """Fold BN into weights; build per-core (half) constant tensors."""
    s1 = inp['bn1_g'] / np.sqrt(inp['bn1_v'] + EPS)
    Wk = s1[:, None] * inp['Wk']
    sh1 = inp['bn1_b'] - inp['bn1_m'] * s1
    s2 = inp['bn2_g'] / np.sqrt(inp['bn2_v'] + EPS)
    W1 = s2[:, None] * inp['W1']
    b1 = s2 * (inp['b1'] - inp['bn2_m']) + inp['bn2_b']
    sv = inp['bnv_g'] / np.sqrt(inp['bnv_v'] + EPS)
    Wv = sv[:, None] * inp['Wv']
    shv = inp['bnv_b'] - inp['bnv_m'] * sv
    W1a, W1b = W1[:, :C], W1[:, C:]
    W2, b2 = inp['W2'], inp['b2']
    gng, gnb = inp['gn_g'], inp['gn_b']

    com = {
        'wkT': _f32(Wk.T),                                   # [128,128]
        'w1aT': _f32(np.tile(W1a.T, (1, 4))),                # [128,128] 4x rep in cols
        'w1bT': _f32(W1b.T),                                 # [128,32]
        'b1p': _f32(np.tile(b1, 4)[:, None]),                # [128,1]
        'w1bTz': _make_w1btz(W1b),                           # [128,4,128]
        'sh1': _f32(sh1[:, None]),                           # [128,1]
    }
    halves = []
    for hf in range(2):
        og0 = hf * 256
        W2h = W2[og0:og0 + 256]          # [256, 32]
        Wvh = Wv[og0:og0 + 256]          # [256, 128]
        b2h = b2[og0:og0 + 256]
        gngh = gng[og0:og0 + 256]
        gnbh = gnb[og0:og0 + 256]
        w2T = np.zeros((128, 2, 128), np.float32)
        wvT = np.zeros((128, 2, 128), np.float32)
        mgT = np.zeros((128, 2, 64), np.float32)
        mb = np.zeros((128, 2, 64), np.float32)
        b2c = np.zeros((128, 2), np.float32)
        shvc = np.zeros((128, 2), np.float32)
        for ch in range(2):
            blk = slice(ch * 128, ch * 128 + 128)
            for j in range(4):
                w2T[j * 32:(j + 1) * 32, ch, :] = W2h[blk].T     # [32m,128og]
            wvT[:, ch, :] = Wvh[blk].T
            b2c[:, ch] = b2h[blk]
            shvc[:, ch] = shv[og0 + ch * 128: og0 + ch * 128 + 128]
            for ogl in range(128):
                o = ch * 32 + ogl // 4                           # local out 0..63
                mgT[ogl, ch, o] = gngh[ch * 128 + ogl]
                mb[ogl, ch, o] = 1.0
        halves.append({
            'w2T': w2T, 'wvT': wvT, 'mgT': mgT, 'mb': mb,
            'b2c': b2c, 'shv': shvc,
            'b2r': _f32(b2h.reshape(1, 2, 128)),
            'gngr': _f32(gngh.reshape(1, 2, 128)),
            'gnbr': _f32(gnbh.reshape(1, 2, 128)),
        })
    return com, halves


def _build_program():
    import concourse.bass as bass
    import concourse.tile as tile
    from concourse import mybir

    f32 = mybir.dt.float32
    bf16 = mybir.dt.bfloat16
    f32r = mybir.dt.float32r
    AF = mybir.ActivationFunctionType
    OP = mybir.AluOpType
    AX = mybir.AxisListType

    nc = bass.Bass()

    di = {}
    def dram_in(name, shape, dt=f32):
        di[name] = nc.dram_tensor(name, shape, dt, kind="ExternalInput").ap()
        return di[name]

    x_d = dram_in('x', [128, HW])
    wkT_d = dram_in('wkT', [128, 128])
    w1aT_d = dram_in('w1aT', [128, 128])
    w1bT_d = dram_in('w1bT', [128, 32])
    w1bTz_d = dram_in('w1bTz', [128, 4, 128])
    b1p_d = dram_in('b1p', [128, 1])
    sh1_d = dram_in('sh1', [128, 1])
    w2T_d = dram_in('w2T', [128, 2, 128])
    wvT_d = dram_in('wvT', [128, 2, 128])
    mgT_d = dram_in('mgT', [128, 2, 64])
    mb_d = dram_in('mb', [128, 2, 64])
    b2c_d = dram_in('b2c', [128, 2])
    shv_d = dram_in('shv', [128, 2])
    b2r_d = dram_in('b2r', [1, 2, 128])
    gngr_d = dram_in('gngr', [1, 2, 128])
    gnbr_d = dram_in('gnbr', [1, 2, 128])
    out_d = nc.dram_tensor('out', [64, HW], f32, kind="ExternalOutput").ap()

    def win(pad_ap, nt, dii, djj):
        """8x56 window of a [P, 58*58] padded buffer, shifted by (dii,djj)."""
        v = pad_ap.rearrange("p (r c) -> p r c", c=PADW)
        r0 = nt * TR
        return v[:, r0 + dii + 1: r0 + dii + 1 + TR, djj + 1: djj + 1 + W]

    with tile.TileContext(nc) as tc:
        with tc.tile_pool(name="const", bufs=1) as cpool, \
             tc.tile_pool(name="big", bufs=1) as bigp, \
             tc.tile_pool(name="work", bufs=3) as wp, \
             tc.tile_pool(name="junkp", bufs=2) as junkp, \
             tc.tile_pool(name="junkp2", bufs=2) as junkp2, \
             tc.tile_pool(name="ps_mm", bufs=2, space="PSUM") as ps_mm, \
             tc.tile_pool(name="ps_a", bufs=4, space="PSUM") as ps_ap, \
             tc.tile_pool(name="ps_av", bufs=2, space="PSUM") as ps_avp:

            # ---- load inputs to SBUF ----
            x_sb = cpool.tile([128, HW], f32, tag="x")
            nc.sync.dma_start(out=x_sb, in_=x_d)
            def ld(ap_d, shape, dt=f32, tag=None):
                t = cpool.tile(shape, dt, tag=tag, name=tag)
                nc.gpsimd.dma_start(out=t, in_=ap_d)
                return t
            def ldv(ap_d, shape, dt=f32, tag=None):
                # stage through a DVE copy so DVE consumers need no sem wait
                t0 = cpool.tile(shape, f32, tag=tag + "_dma", name=tag + "_dma")
                nc.gpsimd.dma_start(out=t0, in_=ap_d)
                t = cpool.tile(shape, dt, tag=tag, name=tag)
                nc.vector.tensor_copy(out=t, in_=t0)
                return t
            wkTf = ld(wkT_d, [128, 128], tag="wkTf")
            w1aTf = ld(w1aT_d, [128, 128], tag="w1aTf")
            w1bTf = ld(w1bT_d, [128, 32], tag="w1bTf")
            wkT = cpool.tile([128, 128], bf16, tag="wkT")
            w1aT = cpool.tile([128, 128], bf16, tag="w1aT")
            w1bT = cpool.tile([128, 32], bf16, tag="w1bT")
            w1bTzf = ld(w1bTz_d, [128, 4, 128], tag="w1bTzf")
            w1bTz = cpool.tile([128, 4, 128], bf16, tag="w1bTz")
            nc.gpsimd.tensor_copy(out=w1bTz, in_=w1bTzf)
            nc.gpsimd.tensor_copy(out=wkT, in_=wkTf)
            nc.gpsimd.tensor_copy(out=w1aT, in_=w1aTf)
            nc.vector.tensor_copy(out=w1bT, in_=w1bTf)
            b1p = ldv(b1p_d, [128, 1], tag="b1p")
            sh1 = ldv(sh1_d, [128, 1], tag="sh1")
            w2Tf = ld(w2T_d, [128, 2, 128], tag="w2Tf")
            wvTf = ld(wvT_d, [128, 2, 128], tag="wvTf")
            wvT = cpool.tile([128, 2, 128], bf16, tag="wvT")
            nc.vector.tensor_copy(out=wvT, in_=wvTf)
            mgTf = ld(mgT_d, [128, 2, 64], tag="mgTf")
            w2T = cpool.tile([128, 2, 128], bf16, tag="w2T")
            mgT = cpool.tile([128, 2, 64], bf16, tag="mgT")
            nc.gpsimd.tensor_copy(out=w2T, in_=w2Tf)
            nc.gpsimd.tensor_copy(out=mgT, in_=mgTf)
            mb = ldv(mb_d, [128, 2, 64], tag="mb")
            b2c = ldv(b2c_d, [128, 2], tag="b2c")
            shv = ldv(shv_d, [128, 2], tag="shv")
            b2r = ldv(b2r_d, [1, 2, 128], tag="b2r")
            gngr = ldv(gngr_d, [1, 2, 128], tag="gngr")
            gnbr = ldv(gnbr_d, [1, 2, 128], tag="gnbr")

            # ---- persistent big buffers ----
            ke_pad = bigp.tile([128, PADHW], bf16, tag="ke_pad")
            h_stack = [bigp.tile([128, HW], bf16, tag=f"hs{g}", name=f"hs{g}") for g in range(3)]
            v_pad = [bigp.tile([128, PADHW], bf16, tag=f"vp{c}", name=f"vp{c}") for c in range(2)]
            v_padf = [bigp.tile([128, PADHW], f32, tag=f"vpf{c}", name=f"vpf{c}") for c in range(2)]
            av_sb = bigp.tile([64, HW], bf16, tag="av")
            w2v_pad = bigp.tile([64, PADHW], bf16, tag="w2vp")
            tmp_pad = bigp.tile([64, PADHW], bf16, tag="tmpp")
            box_sb = bigp.tile([64, HW], bf16, tag="box")
            s2p = bigp.tile([128, 2, 63], f32, tag="s2p")
            def memset_border(buf):
                bv = buf.rearrange("p (r c) -> p r c", c=PADW)
                nc.gpsimd.memset(bv[:, 0:1, :], 0.0)        # top row
                nc.gpsimd.memset(bv[:, PADW - 1:PADW, :], 0.0)  # bottom row
                nc.gpsimd.memset(bv[:, :, 0:1], 0.0)        # left col
                nc.gpsimd.memset(bv[:, :, PADW - 1:PADW], 0.0)  # right col
            for _b in (ke_pad, v_pad[0], v_pad[1], v_padf[0], v_padf[1],
                       w2v_pad, tmp_pad):
                memset_border(_b)

            x_bf = bigp.tile([128, HW], bf16, tag="x_bf")
            nc.vector.tensor_copy(out=x_bf, in_=x_sb)
            xr = x_bf.rearrange("p (t n) -> p t n", t=NT)

            # ---- P1: ke = relu(Wk' x + sh1) -> ke_pad interior (f32) ----
            for nt in range(NT):
                ps = ps_mm.tile([128, TW], f32, tag="ps_mm", name="ps_mm")
                nc.tensor.matmul(ps, wkT, xr[:, nt],
                                 start=True, stop=True)
                nc.scalar.activation(out=win(ke_pad, nt, 0, 0),
                                     in_=ps.rearrange("p (r c) -> p r c", c=W),
                                     func=AF.Relu, bias=sh1, scale=1.0)

            # ---- P2: h groups via PSUM accumulate; relu+b1 -> bf16 ----
            for g, grp in enumerate(PGROUPS):
                hsr = h_stack[g].rearrange("p (t n) -> p t n", t=NT)
                for nt in range(NT):
                    ps = ps_mm.tile([128, TW], f32, tag="ps_mm", name="ps_mm")
                    nc.tensor.matmul(ps, w1aT, xr[:, nt],
                                     start=True, stop=False)
                    for jj, p in enumerate(grp):
                        dii, djj = DELTA[p]
                        nc.tensor.matmul(
                            ps.rearrange("p (r c) -> p r c", c=W),
                            w1bTz[:, jj], win(ke_pad, nt, dii, djj),
                            start=False, stop=(jj == len(grp) - 1))
                    nc.scalar.activation(out=hsr[:, nt], in_=ps,
                                         func=AF.Relu, bias=b1p, scale=1.0)

            # ---- P3: v = Wv' x + shv -> v_pad interior (bf16) ----
            for ch in range(2):
                for nt in range(NT):
                    ps = ps_mm.tile([128, TW], f32, tag="ps_mm", name="ps_mm")
                    nc.tensor.matmul(ps, wvT[:, ch],
                                     xr[:, nt], start=True, stop=True)
                    nc.vector.tensor_scalar(
                        out=win(v_padf[ch], nt, 0, 0),
                        in0=ps.rearrange("p (r c) -> p r c", c=W),
                        scalar1=shv[:, ch:ch + 1], scalar2=None, op0=OP.add)
                    nc.gpsimd.tensor_copy(out=win(v_pad[ch], nt, 0, 0),
                                           in_=win(v_padf[ch], nt, 0, 0))

            # ---- P4: main loop: a0, products, AV, stats partials ----
            avr = av_sb.rearrange("p (t n) -> p t n", t=NT)
            for nt in range(NT):
                ps_av = ps_avp.tile([64, TW], f32, tag="ps_av")
                first = True
                for ch in range(2):
                    for g, grp in enumerate(PGROUPS):
                        hsr = h_stack[g].rearrange("p (t n) -> p t n", t=NT)
                        pstiles = {}
                        for jj, p in enumerate(grp):
                            psa = ps_ap.tile([128, TW], f32, tag="ps_a")
                            nc.tensor.matmul(
                                psa, w2T[jj * 32:(jj + 1) * 32, ch],
                                hsr[jj * 32:(jj + 1) * 32, nt],
                                start=True, stop=True, tile_position=(jj * 32, 0))
                            pstiles[p] = psa
                        for jj, p in enumerate(grp):
                            psa = pstiles[p]
                            scol = nt * K2 + p
                            dii, djj = DELTA[p]
                            t_sb = wp.tile([128, TW], bf16, tag="t_sb", bufs=8)
                            pidx = (nt * 2 + ch) * K2 + p
                            a_sb = wp.tile([128, TW], bf16,
                                           tag="a_sb", bufs=8)
                            if pidx % 12 < 2:
                                nc.vector.tensor_scalar(
                                    out=a_sb, in0=psa, scalar1=0.0,
                                    scalar2=None, op0=OP.add)
                            else:
                                nc.scalar.activation(out=a_sb, in_=psa,
                                                     func=AF.Copy)
                            junk2 = junkp2.tile([128, TW], bf16,
                                                tag="junk2", bufs=6)
                            if pidx % 12 in (2, 3, 4):
                                nc.scalar.activation(
                                    out=junk2, in_=a_sb, func=AF.Square,
                                    accum_out=s2p[:, ch, scol:scol + 1])
                            else:
                                nc.vector.scalar_tensor_tensor(
                                    out=junk2, in0=a_sb, scalar=1.0, in1=a_sb,
                                    op0=OP.mult, op1=OP.mult,
                                    accum_out=s2p[:, ch, scol:scol + 1])
                            nc.gpsimd.tensor_tensor(
                                out=t_sb.rearrange("p (r c) -> p r c", c=W),
                                in0=a_sb.rearrange("p (r c) -> p r c", c=W),
                                in1=win(v_pad[ch], nt, dii, djj),
                                op=OP.mult)
                            # AV += MgT^T @ t
                            nc.tensor.matmul(ps_av, mgT[:, ch], t_sb,
                                             start=first,
                                             stop=(ch == 1 and p == 8))
                            first = False
                # evac AV
                nc.vector.tensor_scalar(out=avr[:, nt], in0=ps_av, scalar1=0.0,
                                        scalar2=None, op0=OP.add)

            # ---- P5: stats -> mu, rstd, w2 weights ----
            S1c = wp.tile([128, 2], f32, tag="S1c")
            S2c = wp.tile([128, 2], f32, tag="S2c")
            hbar3 = wp.tile([128, 4], f32, tag="hbar3")
            nc.vector.memset(hbar3, 0.0)
            nc.vector.tensor_reduce(out=hbar3[:, 0:1], in_=h_stack[0],
                                    axis=AX.X, op=OP.add)
            nc.vector.tensor_reduce(out=hbar3[:, 1:2], in_=h_stack[1],
                                    axis=AX.X, op=OP.add)
            nc.vector.tensor_reduce(out=hbar3[0:32, 2:3], in_=h_stack[2][0:32, :],
                                    axis=AX.X, op=OP.add)
            hbar3b = wp.tile([128, 4], bf16, tag="hbar3b")
            nc.vector.tensor_copy(out=hbar3b, in_=hbar3)
            for ch in range(2):
                s1ps = ps_mm.tile([128, 4], f32, tag="ps_mm", name="ps_mm")
                nc.tensor.matmul(s1ps[:, 0:3], w2T[:, ch], hbar3b[:, 0:3],
                                 start=True, stop=True)
                nc.vector.tensor_reduce(out=S1c[:, ch:ch + 1], in_=s1ps[:, 0:3],
                                        axis=AX.X, op=OP.add)
            nc.vector.tensor_reduce(out=S2c, in_=s2p, axis=AX.X, op=OP.add)
            srow = wp.tile([1, 4, 128], f32, tag="srow")  # S1c0,S1c1,S2c0,S2c1
            for ch in range(2):
                nc.gpsimd.dma_start(out=srow[:, ch, :], in_=S1c[:, ch:ch + 1])
                nc.gpsimd.dma_start(out=srow[:, 2 + ch, :], in_=S2c[:, ch:ch + 1])
            murow = wp.tile([1, 64], f32, tag="murow")
            rsrow = wp.tile([1, 64], f32, tag="rsrow")
            t1 = wp.tile([1, 128], f32, tag="t1")
            t2 = wp.tile([1, 128], f32, tag="t2")
            t3 = wp.tile([1, 128], f32, tag="t3")
            for ch in range(2):
                osl = slice(ch * 32, ch * 32 + 32)
                # mu = mean_gc(S1/N + b2)
                nc.vector.scalar_tensor_tensor(
                    out=t1, in0=srow[:, ch, :], scalar=1.0 / NSTAT,
                    in1=b2r[:, ch], op0=OP.mult, op1=OP.add)
                nc.vector.tensor_reduce(
                    out=murow[:, osl], in_=t1.rearrange("p (o g) -> p o g", g=4),
                    axis=AX.X, op=OP.add)
                nc.vector.tensor_scalar(out=murow[:, osl], in0=murow[:, osl],
                                        scalar1=0.25, scalar2=None, op0=OP.mult)
                # E2 = mean_gc((S2 + 2 b2 S1 + N b2^2)/N)
                nc.vector.scalar_tensor_tensor(
                    out=t2, in0=b2r[:, ch], scalar=2.0, in1=srow[:, ch, :],
                    op0=OP.mult, op1=OP.mult)
                nc.vector.scalar_tensor_tensor(
                    out=t3, in0=b2r[:, ch], scalar=float(NSTAT), in1=b2r[:, ch],
                    op0=OP.mult, op1=OP.mult)
                nc.vector.tensor_tensor(out=t2, in0=t2, in1=t3, op=OP.add)
                nc.vector.tensor_tensor(out=t2, in0=t2, in1=srow[:, 2 + ch, :],
                                        op=OP.add)
                nc.vector.tensor_reduce(
                    out=rsrow[:, osl], in_=t2.rearrange("p (o g) -> p o g", g=4),
                    axis=AX.X, op=OP.add)
                # var = E2/(4N) - mu^2 ; rstd = 1/sqrt(var+eps)
                nc.vector.tensor_scalar(out=rsrow[:, osl], in0=rsrow[:, osl],
                                        scalar1=0.25 / NSTAT, scalar2=None, op0=OP.mult)
                nc.vector.tensor_tensor(out=t3[:, 0:32], in0=murow[:, osl],
                                        in1=murow[:, osl], op=OP.mult)
                nc.vector.tensor_tensor(out=rsrow[:, osl], in0=rsrow[:, osl],
                                        in1=t3[:, 0:32], op=OP.subtract)
            epsT = wp.tile([1, 1], f32, tag="epsT")
            nc.vector.memset(epsT, EPS)
            nc.scalar.activation(out=rsrow, in_=rsrow, func=AF.Sqrt,
                                 bias=epsT, scale=1.0)
            nc.vector.reciprocal(out=rsrow, in_=rsrow)
            # broadcast mu, rstd over gc -> [1,128] per ch; w2 row; DMA to cols
            mub = wp.tile([1, 128], f32, tag="mub")
            rsb = wp.tile([1, 128], f32, tag="rsb")
            w2row = wp.tile([1, 2, 128], f32, tag="w2row")
            w2col = wp.tile([128, 2], f32, tag="w2col")
            rscol = wp.tile([64, 1], f32, tag="rscol")
            for ch in range(2):
                osl = slice(ch * 32, ch * 32 + 32)
                for g in range(4):
                    nc.vector.tensor_copy(
                        out=mub.rearrange("p (o g) -> p o g", g=4)[:, :, g:g + 1],
                        in_=murow[:, osl].rearrange("p (o u) -> p o u", u=1))
                    nc.vector.tensor_copy(
                        out=rsb.rearrange("p (o g) -> p o g", g=4)[:, :, g:g + 1],
                        in_=rsrow[:, osl].rearrange("p (o u) -> p o u", u=1))
                nc.vector.tensor_tensor(out=w2row[:, ch], in0=b2r[:, ch], in1=mub,
                                        op=OP.subtract)
                nc.vector.tensor_tensor(out=w2row[:, ch], in0=w2row[:, ch],
                                        in1=gngr[:, ch], op=OP.mult)
                nc.vector.tensor_tensor(out=w2row[:, ch], in0=w2row[:, ch],
                                        in1=rsb, op=OP.mult)
                nc.vector.tensor_tensor(out=w2row[:, ch], in0=w2row[:, ch],
                                        in1=gnbr[:, ch], op=OP.add)
                nc.gpsimd.dma_start(out=w2col[:, ch:ch + 1], in_=w2row[:, ch])
            nc.gpsimd.dma_start(out=rscol, in_=rsrow)
            w2lhsT = wp.tile([128, 2, 64], bf16, tag="w2lhsT")
            for ch in range(2):
                nc.vector.tensor_scalar(out=w2lhsT[:, ch], in0=mb[:, ch],
                                        scalar1=w2col[:, ch:ch + 1], scalar2=None,
                                        op0=OP.mult)

            # ---- P6: W2V + box ----
            for nt in range(NT):
                ps = ps_mm.tile([64, TW], f32, tag="ps_mm", name="ps_mm")
                nc.tensor.matmul(ps, w2lhsT[:, 0], win(v_pad[0], nt, 0, 0),
                                 start=True, stop=False)
                nc.tensor.matmul(ps, w2lhsT[:, 1], win(v_pad[1], nt, 0, 0),
                                 start=False, stop=True)
                nc.vector.tensor_scalar(out=win(w2v_pad, nt, 0, 0),
                                        in0=ps.rearrange("p (r c) -> p r c", c=W),
                                        scalar1=0.0, scalar2=None, op0=OP.add)
            # horizontal then vertical 3x1 box passes
            wv = w2v_pad.rearrange("p (r c) -> p r c", c=PADW)
            tp = tmp_pad.rearrange("p (r c) -> p r c", c=PADW)
            nc.vector.tensor_tensor(out=tp[:, 1:57, 1:57],
                                    in0=wv[:, 1:57, 0:56], in1=wv[:, 1:57, 2:58],
                                    op=OP.add)
            nc.vector.tensor_tensor(out=tp[:, 1:57, 1:57],
                                    in0=tp[:, 1:57, 1:57], in1=wv[:, 1:57, 1:57],
                                    op=OP.add)
            boxr = box_sb.rearrange("p (r c) -> p r c", c=W)
            nc.vector.tensor_tensor(out=boxr, in0=tp[:, 0:56, 1:57],
                                    in1=tp[:, 2:58, 1:57], op=OP.add)
            nc.vector.tensor_tensor(out=boxr, in0=boxr, in1=tp[:, 1:57, 1:57],
                                    op=OP.add)

            # ---- P7: out = rstd*AV + box ----
            out_sb = bigp.tile([64, HW], f32, tag="out_sb")
            nc.vector.scalar_tensor_tensor(out=out_sb, in0=av_sb, scalar=rscol,
                                           in1=box_sb, op0=OP.mult, op1=OP.add)
            nc.sync.dma_start(out=out_d, in_=out_sb)

    _spill_excess_waits(nc)
    return nc


def _spill_excess_waits(nc):
    """Compute-instruction formats only have room for one inline sync wait.
    Move extra waits onto a same-engine NoOp inserted immediately before the
    instruction — the engine queue executes in order, so waiting there is
    equivalent."""
    from concourse import mybir
    for f in nc.m.functions:
        for bb in f.blocks:
            new = []
            for inst in bb.instructions:
                si = getattr(inst, "sync_info", None)
                if si is not None and si.on_wait and len(si.on_wait) > 1:
                    for k, w in enumerate(si.on_wait[:-1]):
                        new.append(mybir.InstNoOp(
                            name=f"{inst.name}-wspill{k}",
                            ins=[], outs=[],
                            engine=inst.engine,
                            sync_info=mybir.SyncInfo(
                                on_wait=[w], on_update=[]),
                            bass_nofuse=True,
                        ))
                    si.on_wait = [si.on_wait[-1]]
                new.append(inst)
            bb.instructions = new


def _strip_redundant_self_waits(nc):
    """Drop sync waits that are already satisfied by same-engine program
    order: a wait on sem S >= v by engine E is redundant when S is only ever
    incremented by E's own (non-DMA) instructions and E has issued >= v
    increments of S before this instruction. Engines execute their compute
    queue serially, so those increments are guaranteed to have fired.
    Needed because some instruction formats (Matmult) only have room for one
    inline sync-wait command and Tile occasionally emits a redundant
    same-engine wait alongside a real cross-engine one."""
    dma_ops = {"InstDMACopy", "InstDMATranspose", "InstTensorLoad",
               "InstTensorSave", "InstTrigger"}
    seq = []
    for f in nc.m.functions:
        for bb in f.blocks:
            for inst in bb.instructions:
                seq.append(inst)
    upd_engines = {}
    for inst in seq:
        si = getattr(inst, "sync_info", None)
        if si is None:
            continue
        isdma = type(inst).__name__ in dma_ops
        for u in (si.on_update or []):
            key = u.id
            upd_engines.setdefault(key, set()).add(
                "DMA" if isdma else str(inst.engine))
    counts = {}
    for inst in seq:
        si = getattr(inst, "sync_info", None)
        if si is None:
            continue
        eng = str(inst.engine)
        if si.on_wait:
            keep = []
            for w in si.on_wait:
                engs = upd_engines.get(w.id, {"?"})
                seen = counts.get((eng, w.id), 0)
                if (engs == {eng} and w.wait_mode == "sem-ge-imm"
                        and seen >= (w.wait_value or 0)):
                    continue
                keep.append(w)
            if len(keep) != len(si.on_wait):
                si.on_wait = keep
        if type(inst).__name__ not in dma_ops:
            for u in (si.on_update or []):
                counts[(eng, u.id)] = counts.get((eng, u.id), 0) + (
                    u.update_value or 1)


_CACHE = {}


def kernel(**inputs):
    inputs = {k: np.asarray(v) for k, v in inputs.items()}
    com, halves = _prep_weights(inputs)
    x = inputs['x'].astype(np.float32)

    if 'nc' not in _CACHE:
        _CACHE['nc'] = _build_program()
    nc = _CACHE['nc']

    in_maps = []
    for core in range(8):
        b, hf = core // 2, core % 2
        m = dict(com)
        hv = halves[hf]
        m['x'] = _f32(x[b].reshape(128, HW))
        for k in ('w2T', 'mgT', 'wvT', 'mb', 'b2c', 'shv', 'b2r', 'gngr', 'gnbr'):
            m[k] = _f32(hv[k])
        in_maps.append(m)

    from concourse import bass_utils
    res = bass_utils.run_bass_kernel_spmd(nc, in_maps, core_ids=list(range(8)))
    out = np.zeros((B, COUT, H, W), np.float32)
    for core in range(8):
        b, hf = core // 2, core % 2
        out[b, hf * 64:(hf + 1) * 64] = res.results[core]['out'].reshape(64, H, W)
    return out


# revision 42
# speedup vs baseline: 1.0118x; 1.0118x over previous
"""Trainium2 Bass kernel for the 3x3 local-attention module (sparse_attention).

Sharding: 8 cores = (batch b in 0..3) x (out-channel half in 0..1).
Each core computes one batch element and 64 of the 128 output channels
(= 256 of the 512 OG channels). No cross-core communication; GroupNorm
statistics are exact per (b, out) and live entirely on one core.

Math restructure (validated in numpy against the jax reference, ~4e-3 rel l2):
  ke  = relu(Wk' x + sh1)                  (BN folded into Wk)
  h_p = relu(W1a x + W1b shift_p(ke) + b1')   via PSUM accumulation
  a0  = W2 h                               (no bias; b2 handled analytically)
  stats: S1 = sum a0, S2 = sum a0^2 per og; group mean/var derived with b2.
  AV[out,hw] = sum_{gc,p} gn_g * a0 * shift_p(v)   (products on DVE/GPSIMD,
               gc-sum + gn_g weights + p-sum via one PE matmul per (chunk,p))
  const term = box3x3( sum_gc w2[og]*v[og] ),  w2 = gn_g*(b2-mu)*rstd + gn_b
  out = rstd * AV + const
"""

import numpy as np

B, C, H, W = 4, 128, 56, 56
HW = H * W
K2 = 9
MID = 32
OG = 512
COUT = 128
GC = 4
EPS = 1e-5
NT = 7                # N-tiles over HW
TW = HW // NT         # 448 = 8 rows of 56
TR = TW // W          # 8 rows per tile
PADW = W + 2          # 58
PADHW = PADW * PADW   # 3364
NSTAT = K2 * HW       # per-og element count for stats
DELTA = [(i - 1, j - 1) for i in range(3) for j in range(3)]
PGROUPS = [[0, 1, 2, 3], [4, 5, 6, 7], [8]]


def _f32(x):
    return np.ascontiguousarray(x, dtype=np.float32)


def _make_w1btz(W1b):
    z = np.zeros((128, 4, 128), np.float32)
    for j in range(4):
        z[:, j, j * 32:(j + 1) * 32] = W1b.T
    return z


def _prep_weights(inp):
    """Fold BN into weights; build per-core (half) constant tensors."""
    s1 = inp['bn1_g'] / np.sqrt(inp['bn1_v'] + EPS)
    Wk = s1[:, None] * inp['Wk']
    sh1 = inp['bn1_b'] - inp['bn1_m'] * s1
    s2 = inp['bn2_g'] / np.sqrt(inp['bn2_v'] + EPS)
    W1 = s2[:, None] * inp['W1']
    b1 = s2 * (inp['b1'] - inp['bn2_m']) + inp['bn2_b']
    sv = inp['bnv_g'] / np.sqrt(inp['bnv_v'] + EPS)
    Wv = sv[:, None] * inp['Wv']
    shv = inp['bnv_b'] - inp['bnv_m'] * sv
    W1a, W1b = W1[:, :C], W1[:, C:]
    W2, b2 = inp['W2'], inp['b2']
    gng, gnb = inp['gn_g'], inp['gn_b']

    com = {
        'wkT': _f32(Wk.T),                                   # [128,128]
        'w1aT': _f32(np.tile(W1a.T, (1, 4))),                # [128,128] 4x rep in cols
        'w1bT': _f32(W1b.T),                                 # [128,32]
        'b1p': _f32(np.tile(b1, 4)[:, None]),                # [128,1]
        'w1bTz': _make_w1btz(W1b),                           # [128,4,128]
        'sh1': _f32(sh1[:, None]),                           # [128,1]
    }
    halves = []
    for hf in range(2):
        og0 = hf * 256
        W2h = W2[og0:og0 + 256]          # [256, 32]
        Wvh = Wv[og0:og0 + 256]          # [256, 128]
        b2h = b2[og0:og0 + 256]
        gngh = gng[og0:og0 + 256]
        gnbh = gnb[og0:og0 + 256]
        w2T = np.zeros((128, 2, 128), np.float32)
        wvT = np.zeros((128, 2, 128), np.float32)
        mgT = np.zeros((128, 2, 64), np.float32)
        mb = np.zeros((128, 2, 64), np.float32)
        b2c = np.zeros((128, 2), np.float32)
        shvc = np.zeros((128, 2), np.float32)
        for ch in range(2):
            blk = slice(ch * 128, ch * 128 + 128)
            for j in range(4):
                w2T[j * 32:(j + 1) * 32, ch, :] = W2h[blk].T     # [32m,128og]
            wvT[:, ch, :] = Wvh[blk].T
            b2c[:, ch] = b2h[blk]
            shvc[:, ch] = shv[og0 + ch * 128: og0 + ch * 128 + 128]
            for ogl in range(128):
                o = ch * 32 + ogl // 4                           # local out 0..63
                mgT[ogl, ch, o] = gngh[ch * 128 + ogl]
                mb[ogl, ch, o] = 1.0
        halves.append({
            'w2T': w2T, 'wvT': wvT, 'mgT': mgT, 'mb': mb,
            'b2c': b2c, 'shv': shvc,
            'b2r': _f32(b2h.reshape(1, 2, 128)),
            'gngr': _f32(gngh.reshape(1, 2, 128)),
            'gnbr': _f32(gnbh.reshape(1, 2, 128)),
        })
    return com, halves


def _build_program():
    import concourse.bass as bass
    import concourse.tile as tile
    from concourse import mybir

    f32 = mybir.dt.float32
    bf16 = mybir.dt.bfloat16
    f32r = mybir.dt.float32r
    AF = mybir.ActivationFunctionType
    OP = mybir.AluOpType
    AX = mybir.AxisListType

    nc = bass.Bass()

    di = {}
    def dram_in(name, shape, dt=f32):
        di[name] = nc.dram_tensor(name, shape, dt, kind="ExternalInput").ap()
        return di[name]

    x_d = dram_in('x', [128, HW])
    wkT_d = dram_in('wkT', [128, 128])
    w1aT_d = dram_in('w1aT', [128, 128])
    w1bT_d = dram_in('w1bT', [128, 32])
    w1bTz_d = dram_in('w1bTz', [128, 4, 128])
    b1p_d = dram_in('b1p', [128, 1])
    sh1_d = dram_in('sh1', [128, 1])
    w2T_d = dram_in('w2T', [128, 2, 128])
    wvT_d = dram_in('wvT', [128, 2, 128])
    mgT_d = dram_in('mgT', [128, 2, 64])
    mb_d = dram_in('mb', [128, 2, 64])
    b2c_d = dram_in('b2c', [128, 2])
    shv_d = dram_in('shv', [128, 2])
    b2r_d = dram_in('b2r', [1, 2, 128])
    gngr_d = dram_in('gngr', [1, 2, 128])
    gnbr_d = dram_in('gnbr', [1, 2, 128])
    out_d = nc.dram_tensor('out', [64, HW], f32, kind="ExternalOutput").ap()

    def win(pad_ap, nt, dii, djj):
        """8x56 window of a [P, 58*58] padded buffer, shifted by (dii,djj)."""
        v = pad_ap.rearrange("p (r c) -> p r c", c=PADW)
        r0 = nt * TR
        return v[:, r0 + dii + 1: r0 + dii + 1 + TR, djj + 1: djj + 1 + W]

    with tile.TileContext(nc) as tc:
        with tc.tile_pool(name="const", bufs=1) as cpool, \
             tc.tile_pool(name="big", bufs=1) as bigp, \
             tc.tile_pool(name="work", bufs=3) as wp, \
             tc.tile_pool(name="junkp", bufs=2) as junkp, \
             tc.tile_pool(name="junkp2", bufs=2) as junkp2, \
             tc.tile_pool(name="ps_mm", bufs=2, space="PSUM") as ps_mm, \
             tc.tile_pool(name="ps_a", bufs=4, space="PSUM") as ps_ap, \
             tc.tile_pool(name="ps_av", bufs=2, space="PSUM") as ps_avp:

            # ---- load inputs to SBUF ----
            x_sb = cpool.tile([128, HW], f32, tag="x")
            nc.sync.dma_start(out=x_sb, in_=x_d)
            def ld(ap_d, shape, dt=f32, tag=None):
                t = cpool.tile(shape, dt, tag=tag, name=tag)
                nc.gpsimd.dma_start(out=t, in_=ap_d)
                return t
            def ldv(ap_d, shape, dt=f32, tag=None):
                # stage through a DVE copy so DVE consumers need no sem wait
                t0 = cpool.tile(shape, f32, tag=tag + "_dma", name=tag + "_dma")
                nc.gpsimd.dma_start(out=t0, in_=ap_d)
                t = cpool.tile(shape, dt, tag=tag, name=tag)
                nc.vector.tensor_copy(out=t, in_=t0)
                return t
            wkTf = ld(wkT_d, [128, 128], tag="wkTf")
            w1aTf = ld(w1aT_d, [128, 128], tag="w1aTf")
            w1bTf = ld(w1bT_d, [128, 32], tag="w1bTf")
            wkT = cpool.tile([128, 128], bf16, tag="wkT")
            w1aT = cpool.tile([128, 128], bf16, tag="w1aT")
            w1bT = cpool.tile([128, 32], bf16, tag="w1bT")
            w1bTzf = ld(w1bTz_d, [128, 4, 128], tag="w1bTzf")
            w1bTz = cpool.tile([128, 4, 128], bf16, tag="w1bTz")
            nc.gpsimd.tensor_copy(out=w1bTz, in_=w1bTzf)
            nc.gpsimd.tensor_copy(out=wkT, in_=wkTf)
            nc.gpsimd.tensor_copy(out=w1aT, in_=w1aTf)
            nc.vector.tensor_copy(out=w1bT, in_=w1bTf)
            b1p = ldv(b1p_d, [128, 1], tag="b1p")
            sh1 = ldv(sh1_d, [128, 1], tag="sh1")
            w2Tf = ld(w2T_d, [128, 2, 128], tag="w2Tf")
            wvTf = ld(wvT_d, [128, 2, 128], tag="wvTf")
            wvT = cpool.tile([128, 2, 128], bf16, tag="wvT")
            nc.vector.tensor_copy(out=wvT, in_=wvTf)
            mgTf = ld(mgT_d, [128, 2, 64], tag="mgTf")
            w2T = cpool.tile([128, 2, 128], bf16, tag="w2T")
            mgT = cpool.tile([128, 2, 64], bf16, tag="mgT")
            nc.gpsimd.tensor_copy(out=w2T, in_=w2Tf)
            nc.gpsimd.tensor_copy(out=mgT, in_=mgTf)
            mb = ldv(mb_d, [128, 2, 64], tag="mb")
            b2c = ldv(b2c_d, [128, 2], tag="b2c")
            shv = ldv(shv_d, [128, 2], tag="shv")
            b2r = ldv(b2r_d, [1, 2, 128], tag="b2r")
            gngr = ldv(gngr_d, [1, 2, 128], tag="gngr")
            gnbr = ldv(gnbr_d, [1, 2, 128], tag="gnbr")

            # ---- persistent big buffers ----
            ke_pad = bigp.tile([128, PADHW], bf16, tag="ke_pad")
            h_stack = [bigp.tile([128, HW], bf16, tag=f"hs{g}", name=f"hs{g}") for g in range(3)]
            v_pad = [bigp.tile([128, PADHW], bf16, tag=f"vp{c}", name=f"vp{c}") for c in range(2)]
            v_padf = [bigp.tile([128, PADHW], f32, tag=f"vpf{c}", name=f"vpf{c}") for c in range(2)]
            av_sb = bigp.tile([64, HW], bf16, tag="av")
            w2v_pad = bigp.tile([64, PADHW], bf16, tag="w2vp")
            tmp_pad = bigp.tile([64, PADHW], bf16, tag="tmpp")
            box_sb = bigp.tile([64, HW], bf16, tag="box")
            s2p = bigp.tile([128, 2, 63], f32, tag="s2p")
            def memset_border(buf):
                bv = buf.rearrange("p (r c) -> p r c", c=PADW)
                nc.gpsimd.memset(bv[:, 0:1, :], 0.0)        # top row
                nc.gpsimd.memset(bv[:, PADW - 1:PADW, :], 0.0)  # bottom row
                nc.gpsimd.memset(bv[:, :, 0:1], 0.0)        # left col
                nc.gpsimd.memset(bv[:, :, PADW - 1:PADW], 0.0)  # right col
            for _b in (ke_pad, v_pad[0], v_pad[1], v_padf[0], v_padf[1],
                       w2v_pad, tmp_pad):
                memset_border(_b)

            x_bf = bigp.tile([128, HW], bf16, tag="x_bf")
            nc.vector.tensor_copy(out=x_bf, in_=x_sb)
            xr = x_bf.rearrange("p (t n) -> p t n", t=NT)

            # ---- P1: ke = relu(Wk' x + sh1) -> ke_pad interior (f32) ----
            for nt in range(NT):
                ps = ps_mm.tile([128, TW], f32, tag="ps_mm", name="ps_mm")
                nc.tensor.matmul(ps, wkT, xr[:, nt],
                                 start=True, stop=True)
                nc.scalar.activation(out=win(ke_pad, nt, 0, 0),
                                     in_=ps.rearrange("p (r c) -> p r c", c=W),
                                     func=AF.Relu, bias=sh1, scale=1.0)

            # ---- P2: h groups via PSUM accumulate; relu+b1 -> bf16 ----
            for g, grp in enumerate(PGROUPS):
                hsr = h_stack[g].rearrange("p (t n) -> p t n", t=NT)
                for nt in range(NT):
                    ps = ps_mm.tile([128, TW], f32, tag="ps_mm", name="ps_mm")
                    nc.tensor.matmul(ps, w1aT, xr[:, nt],
                                     start=True, stop=False)
                    for jj, p in enumerate(grp):
                        dii, djj = DELTA[p]
                        nc.tensor.matmul(
                            ps.rearrange("p (r c) -> p r c", c=W),
                            w1bTz[:, jj], win(ke_pad, nt, dii, djj),
                            start=False, stop=(jj == len(grp) - 1))
                    nc.scalar.activation(out=hsr[:, nt], in_=ps,
                                         func=AF.Relu, bias=b1p, scale=1.0)

            # ---- P3: v = Wv' x + shv -> v_pad interior (bf16) ----
            for ch in range(2):
                for nt in range(NT):
                    ps = ps_mm.tile([128, TW], f32, tag="ps_mm", name="ps_mm")
                    nc.tensor.matmul(ps, wvT[:, ch],
                                     xr[:, nt], start=True, stop=True)
                    nc.vector.tensor_scalar(
                        out=win(v_padf[ch], nt, 0, 0),
                        in0=ps.rearrange("p (r c) -> p r c", c=W),
                        scalar1=shv[:, ch:ch + 1], scalar2=None, op0=OP.add)
                    nc.gpsimd.tensor_copy(out=win(v_pad[ch], nt, 0, 0),
                                           in_=win(v_padf[ch], nt, 0, 0))

            # ---- P4: main loop: a0, products, AV, stats partials ----
            avr = av_sb.rearrange("p (t n) -> p t n", t=NT)
            for nt in range(NT):
                ps_av = ps_avp.tile([64, TW], f32, tag="ps_av")
                first = True
                for ch in range(2):
                    for g, grp in enumerate(PGROUPS):
                        hsr = h_stack[g].rearrange("p (t n) -> p t n", t=NT)
                        pstiles = {}
                        for jj, p in enumerate(grp):
                            psa = ps_ap.tile([128, TW], f32, tag="ps_a")
                            nc.tensor.matmul(
                                psa, w2T[jj * 32:(jj + 1) * 32, ch],
                                hsr[jj * 32:(jj + 1) * 32, nt],
                                start=True, stop=True, tile_position=(jj * 32, 0))
                            pstiles[p] = psa
                        for jj, p in enumerate(grp):
                            psa = pstiles[p]
                            scol = nt * K2 + p
                            dii, djj = DELTA[p]
                            t_sb = wp.tile([128, TW], bf16, tag="t_sb", bufs=8)
                            pidx = (nt * 2 + ch) * K2 + p
                            a_sb = wp.tile([128, TW], bf16,
                                           tag="a_sb", bufs=8)
                            if pidx % 12 < 2:
                                nc.vector.tensor_scalar(
                                    out=a_sb, in0=psa, scalar1=0.0,
                                    scalar2=None, op0=OP.add)
                            else:
                                nc.scalar.activation(out=a_sb, in_=psa,
                                                     func=AF.Copy)
                            junk2 = junkp2.tile([128, TW], bf16,
                                                tag="junk2", bufs=6)
                            if pidx % 12 in (2, 3, 4):
                                nc.scalar.activation(
                                    out=junk2, in_=a_sb, func=AF.Square,
                                    accum_out=s2p[:, ch, scol:scol + 1])
                            else:
                                nc.vector.scalar_tensor_tensor(
                                    out=junk2, in0=a_sb, scalar=1.0, in1=a_sb,
                                    op0=OP.mult, op1=OP.mult,
                                    accum_out=s2p[:, ch, scol:scol + 1])
                            nc.gpsimd.tensor_tensor(
                                out=t_sb.rearrange("p (r c) -> p r c", c=W),
                                in0=a_sb.rearrange("p (r c) -> p r c", c=W),
                                in1=win(v_pad[ch], nt, dii, djj),
                                op=OP.mult)
                            # AV += MgT^T @ t
                            nc.tensor.matmul(ps_av, mgT[:, ch], t_sb,
                                             start=first,
                                             stop=(ch == 1 and p == 8))
                            first = False
                # evac AV
                nc.vector.tensor_scalar(out=avr[:, nt], in0=ps_av, scalar1=0.0,
                                        scalar2=None, op0=OP.add)

            # ---- P5: stats -> mu, rstd, w2 weights ----
            S1c = wp.tile([128, 2], f32, tag="S1c")
            S2c = wp.tile([128, 2], f32, tag="S2c")
            hbar3 = wp.tile([128, 4], f32, tag="hbar3")
            nc.vector.memset(hbar3, 0.0)
            nc.vector.tensor_reduce(out=hbar3[:, 0:1], in_=h_stack[0],
                                    axis=AX.X, op=OP.add)
            nc.vector.tensor_reduce(out=hbar3[:, 1:2], in_=h_stack[1],
                                    axis=AX.X, op=OP.add)
            nc.vector.tensor_reduce(out=hbar3[0:32, 2:3], in_=h_stack[2][0:32, :],
                                    axis=AX.X, op=OP.add)
            hbar3b = wp.tile([128, 4], bf16, tag="hbar3b")
            nc.vector.tensor_copy(out=hbar3b, in_=hbar3)
            for ch in range(2):
                s1ps = ps_mm.tile([128, 4], f32, tag="ps_mm", name="ps_mm")
                nc.tensor.matmul(s1ps[:, 0:3], w2T[:, ch], hbar3b[:, 0:3],
                                 start=True, stop=True)
                nc.vector.tensor_reduce(out=S1c[:, ch:ch + 1], in_=s1ps[:, 0:3],
                                        axis=AX.X, op=OP.add)
            nc.vector.tensor_reduce(out=S2c, in_=s2p, axis=AX.X, op=OP.add)
            srow = wp.tile([1, 4, 128], f32, tag="srow")  # S1c0,S1c1,S2c0,S2c1
            for ch in range(2):
                nc.gpsimd.dma_start(out=srow[:, ch, :], in_=S1c[:, ch:ch + 1])
                nc.gpsimd.dma_start(out=srow[:, 2 + ch, :], in_=S2c[:, ch:ch + 1])
            murow = wp.tile([1, 64], f32, tag="murow")
            rsrow = wp.tile([1, 64], f32, tag="rsrow")
            t1 = wp.tile([1, 128], f32, tag="t1")
            t2 = wp.tile([1, 128], f32, tag="t2")
            t3 = wp.tile([1, 128], f32, tag="t3")
            for ch in range(2):
                osl = slice(ch * 32, ch * 32 + 32)
                # mu = mean_gc(S1/N + b2)
                nc.vector.scalar_tensor_tensor(
                    out=t1, in0=srow[:, ch, :], scalar=1.0 / NSTAT,
                    in1=b2r[:, ch], op0=OP.mult, op1=OP.add)
                nc.vector.tensor_reduce(
                    out=murow[:, osl], in_=t1.rearrange("p (o g) -> p o g", g=4),
                    axis=AX.X, op=OP.add)
                nc.vector.tensor_scalar(out=murow[:, osl], in0=murow[:, osl],
                                        scalar1=0.25, scalar2=None, op0=OP.mult)
                # E2 = mean_gc((S2 + 2 b2 S1 + N b2^2)/N)
                nc.vector.scalar_tensor_tensor(
                    out=t2, in0=b2r[:, ch], scalar=2.0, in1=srow[:, ch, :],
                    op0=OP.mult, op1=OP.mult)
                nc.vector.scalar_tensor_tensor(
                    out=t3, in0=b2r[:, ch], scalar=float(NSTAT), in1=b2r[:, ch],
                    op0=OP.mult, op1=OP.mult)
                nc.vector.tensor_tensor(out=t2, in0=t2, in1=t3, op=OP.add)
                nc.vector.tensor_tensor(out=t2, in0=t2, in1=srow[:, 2 + ch, :],
                                        op=OP.add)
                nc.vector.tensor_reduce(
                    out=rsrow[:, osl], in_=t2.rearrange("p (o g) -> p o g", g=4),
                    axis=AX.X, op=OP.add)
                # var = E2/(4N) - mu^2 ; rstd = 1/sqrt(var+eps)
                nc.vector.tensor_scalar(out=rsrow[:, osl], in0=rsrow[:, osl],
                                        scalar1=0.25 / NSTAT, scalar2=None, op0=OP.mult)
                nc.vector.tensor_tensor(out=t3[:, 0:32], in0=murow[:, osl],
                                        in1=murow[:, osl], op=OP.mult)
                nc.vector.tensor_tensor(out=rsrow[:, osl], in0=rsrow[:, osl],
                                        in1=t3[:, 0:32], op=OP.subtract)
            epsT = wp.tile([1, 1], f32, tag="epsT")
            nc.vector.memset(epsT, EPS)
            nc.scalar.activation(out=rsrow, in_=rsrow, func=AF.Sqrt,
                                 bias=epsT, scale=1.0)
            nc.vector.reciprocal(out=rsrow, in_=rsrow)
            # broadcast mu, rstd over gc -> [1,128] per ch; w2 row; DMA to cols
            mub = wp.tile([1, 128], f32, tag="mub")
            rsb = wp.tile([1, 128], f32, tag="rsb")
            w2row = wp.tile([1, 2, 128], f32, tag="w2row")
            w2col = wp.tile([128, 2], f32, tag="w2col")
            rscol = wp.tile([64, 1], f32, tag="rscol")
            for ch in range(2):
                osl = slice(ch * 32, ch * 32 + 32)
                for g in range(4):
                    nc.vector.tensor_copy(
                        out=mub.rearrange("p (o g) -> p o g", g=4)[:, :, g:g + 1],
                        in_=murow[:, osl].rearrange("p (o u) -> p o u", u=1))
                    nc.vector.tensor_copy(
                        out=rsb.rearrange("p (o g) -> p o g", g=4)[:, :, g:g + 1],
                        in_=rsrow[:, osl].rearrange("p (o u) -> p o u", u=1))
                nc.vector.tensor_tensor(out=w2row[:, ch], in0=b2r[:, ch], in1=mub,
                                        op=OP.subtract)
                nc.vector.tensor_tensor(out=w2row[:, ch], in0=w2row[:, ch],
                                        in1=gngr[:, ch], op=OP.mult)
                nc.vector.tensor_tensor(out=w2row[:, ch], in0=w2row[:, ch],
                                        in1=rsb, op=OP.mult)
                nc.vector.tensor_tensor(out=w2row[:, ch], in0=w2row[:, ch],
                                        in1=gnbr[:, ch], op=OP.add)
                nc.gpsimd.dma_start(out=w2col[:, ch:ch + 1], in_=w2row[:, ch])
            nc.gpsimd.dma_start(out=rscol, in_=rsrow)
            w2lhsT = wp.tile([128, 2, 64], bf16, tag="w2lhsT")
            for ch in range(2):
                nc.vector.tensor_scalar(out=w2lhsT[:, ch], in0=mb[:, ch],
                                        scalar1=w2col[:, ch:ch + 1], scalar2=None,
                                        op0=OP.mult)

            # ---- P6: W2V + box ----
            for nt in range(NT):
                ps = ps_mm.tile([64, TW], f32, tag="ps_mm", name="ps_mm")
                nc.tensor.matmul(ps, w2lhsT[:, 0], win(v_pad[0], nt, 0, 0),
                                 start=True, stop=False)
                nc.tensor.matmul(ps, w2lhsT[:, 1], win(v_pad[1], nt, 0, 0),
                                 start=False, stop=True)
                nc.scalar.activation(out=win(w2v_pad, nt, 0, 0),
                                     in_=ps.rearrange("p (r c) -> p r c", c=W),
                                     func=AF.Copy)
            # horizontal then vertical 3x1 box passes
            wv = w2v_pad.rearrange("p (r c) -> p r c", c=PADW)
            tp = tmp_pad.rearrange("p (r c) -> p r c", c=PADW)
            nc.vector.tensor_tensor(out=tp[:, 1:57, 1:57],
                                    in0=wv[:, 1:57, 0:56], in1=wv[:, 1:57, 2:58],
                                    op=OP.add)
            nc.vector.tensor_tensor(out=tp[:, 1:57, 1:57],
                                    in0=tp[:, 1:57, 1:57], in1=wv[:, 1:57, 1:57],
                                    op=OP.add)
            boxr = box_sb.rearrange("p (r c) -> p r c", c=W)
            nc.vector.tensor_tensor(out=boxr, in0=tp[:, 0:56, 1:57],
                                    in1=tp[:, 2:58, 1:57], op=OP.add)
            nc.vector.tensor_tensor(out=boxr, in0=boxr, in1=tp[:, 1:57, 1:57],
                                    op=OP.add)

            # ---- P7: out = rstd*AV + box ----
            out_sb = bigp.tile([64, HW], f32, tag="out_sb")
            HH = HW // 2
            for hh in range(2):
                sl = slice(hh * HH, hh * HH + HH)
                nc.vector.scalar_tensor_tensor(
                    out=out_sb[:, sl], in0=av_sb[:, sl], scalar=rscol,
                    in1=box_sb[:, sl], op0=OP.mult, op1=OP.add)
                nc.sync.dma_start(out=out_d[:, sl], in_=out_sb[:, sl])

    _spill_excess_waits(nc)
    return nc


def _spill_excess_waits(nc):
    """Compute-instruction formats only have room for one inline sync wait.
    Move extra waits onto a same-engine NoOp inserted immediately before the
    instruction — the engine queue executes in order, so waiting there is
    equivalent."""
    from concourse import mybir
    for f in nc.m.functions:
        for bb in f.blocks:
            new = []
            for inst in bb.instructions:
                si = getattr(inst, "sync_info", None)
                if si is not None and si.on_wait and len(si.on_wait) > 1:
                    for k, w in enumerate(si.on_wait[:-1]):
                        new.append(mybir.InstNoOp(
                            name=f"{inst.name}-wspill{k}",
                            ins=[], outs=[],
                            engine=inst.engine,
                            sync_info=mybir.SyncInfo(
                                on_wait=[w], on_update=[]),
                            bass_nofuse=True,
                        ))
                    si.on_wait = [si.on_wait[-1]]
                new.append(inst)
            bb.instructions = new


def _strip_redundant_self_waits(nc):
    """Drop sync waits that are already satisfied by same-engine program
    order: a wait on sem S >= v by engine E is redundant when S is only ever
    incremented by E's own (non-DMA) instructions and E has issued >= v
    increments of S before this instruction. Engines execute their compute
    queue serially, so those increments are guaranteed to have fired.
    Needed because some instruction formats (Matmult) only have room for one
    inline sync-wait command and Tile occasionally emits a redundant
    same-engine wait alongside a real cross-engine one."""
    dma_ops = {"InstDMACopy", "InstDMATranspose", "InstTensorLoad",
               "InstTensorSave", "InstTrigger"}
    seq = []
    for f in nc.m.functions:
        for bb in f.blocks:
            for inst in bb.instructions:
                seq.append(inst)
    upd_engines = {}
    for inst in seq:
        si = getattr(inst, "sync_info", None)
        if si is None:
            continue
        isdma = type(inst).__name__ in dma_ops
        for u in (si.on_update or []):
            key = u.id
            upd_engines.setdefault(key, set()).add(
                "DMA" if isdma else str(inst.engine))
    counts = {}
    for inst in seq:
        si = getattr(inst, "sync_info", None)
        if si is None:
            continue
        eng = str(inst.engine)
        if si.on_wait:
            keep = []
            for w in si.on_wait:
                engs = upd_engines.get(w.id, {"?"})
                seen = counts.get((eng, w.id), 0)
                if (engs == {eng} and w.wait_mode == "sem-ge-imm"
                        and seen >= (w.wait_value or 0)):
                    continue
                keep.append(w)
            if len(keep) != len(si.on_wait):
                si.on_wait = keep
        if type(inst).__name__ not in dma_ops:
            for u in (si.on_update or []):
                counts[(eng, u.id)] = counts.get((eng, u.id), 0) + (
                    u.update_value or 1)


_CACHE = {}


def kernel(**inputs):
    inputs = {k: np.asarray(v) for k, v in inputs.items()}
    com, halves = _prep_weights(inputs)
    x = inputs['x'].astype(np.float32)

    if 'nc' not in _CACHE:
        _CACHE['nc'] = _build_program()
    nc = _CACHE['nc']

    in_maps = []
    for core in range(8):
        b, hf = core // 2, core % 2
        m = dict(com)
        hv = halves[hf]
        m['x'] = _f32(x[b].reshape(128, HW))
        for k in ('w2T', 'mgT', 'wvT', 'mb', 'b2c', 'shv', 'b2r', 'gngr', 'gnbr'):
            m[k] = _f32(hv[k])
        in_maps.append(m)

    from concourse import bass_utils
    res = bass_utils.run_bass_kernel_spmd(nc, in_maps, core_ids=list(range(8)))
    out = np.zeros((B, COUT, H, W), np.float32)
    for core in range(8):
        b, hf = core // 2, core % 2
        out[b, hf * 64:(hf + 1) * 64] = res.results[core]['out'].reshape(64, H, W)
    return out
